# revision 38
# baseline (speedup 1.0000x reference)
"""Multi-head attention (B=8, N=1024, C=1024, H=16) on 8 TRN2 NeuronCores.

Strategy: pure data parallelism — each core computes one batch element with
replicated weights (no collectives). Per-core single-head-dim layout:

  inputs (host-prepped, transposed so every matmul contracts on partitions):
    xT   [C, N]      = x[b].T
    wqkT [C, 2C_qk]  = in_proj_weight[0:2048].T     (q then k features)
    wvT  [C, C]      = in_proj_weight[2048:3072].T
    woT  [C, C]      = out_proj_weight.T            ((h,d) rows, co cols)
  phases on-device (all matmuls in float32r: full-rate fp32, ~1e-3 rounding):
    A: V natural [token, vfeat] per 65-wide head group w/ ones column
       (the ones column makes the PV matmul also produce softmax row-sums)
    B: qkT [feature, token] (transposed q/k for QK^T)
    C: per head h: S^T[key, query] = K_h^T.T @ Q_h^T; P = exp(S*scale);
       O^T[d+1, query] = V_hat.T @ P^T (row 64 = softmax sums);
       normalize via reciprocal + gpsimd partition_broadcast
    D: outT [co, token] = woT.T @ A^T + bias

Output: outT per core, host transposes back and stacks.
"""
import numpy as np

B, N, C = 8, 1024, 1024
H = 16
HD = C // H               # 64
SCALE = HD ** (-0.5)
NCORES = 8

_COMPILED = {}


def _build():
    import concourse.bass as bass
    import concourse.tile as tile
    from concourse import bacc, mybir

    F32 = mybir.dt.float32
    F32R = mybir.dt.float32r
    EXP = mybir.ActivationFunctionType.Exp

    nc = bacc.Bacc("TRN2", target_bir_lowering=False, debug=False)

    xT = nc.dram_tensor("xT", [C, N], F32R, kind="ExternalInput").ap()
    wqkT = nc.dram_tensor("wqkT", [C, 2 * C], F32R, kind="ExternalInput").ap()
    wvT = nc.dram_tensor("wvT", [C, C], F32R, kind="ExternalInput").ap()
    woT = nc.dram_tensor("woT", [C, C], F32R, kind="ExternalInput").ap()
    bqk = nc.dram_tensor("bqk", [128, 16], F32, kind="ExternalInput").ap()
    bv = nc.dram_tensor("bv", [1, C], F32R, kind="ExternalInput").ap()
    bo = nc.dram_tensor("bo", [128, 8], F32, kind="ExternalInput").ap()
    ones_col = nc.dram_tensor("ones_col", [128, 16], F32R, kind="ExternalInput").ap()
    ones_row = nc.dram_tensor("ones_row", [1, 512], F32R, kind="ExternalInput").ap()
    outT = nc.dram_tensor("outT", [C, N], F32, kind="ExternalOutput").ap()

    CB = C // 128      # 8 contraction blocks
    TB = N // 128      # 8 token blocks
    JB = 2 * C // 128  # 16 qk feature blocks
    VW = 65            # per-head V width (64 feats + ones col)

    with tile.TileContext(nc) as tc:
        with tc.tile_pool(name="misc", bufs=1) as pool_misc, \
             tc.tile_pool(name="V", bufs=1) as pool_V, \
             tc.tile_pool(name="qk", bufs=1) as pool_qk:

            bqk_sb = pool_misc.tile([128, 16], F32, tag="bqk")
            bv_sb = pool_misc.tile([1, C], F32R, tag="bv")
            bo_sb = pool_misc.tile([128, 8], F32, tag="bo")
            ones_sb = pool_misc.tile([1, 512], F32R, tag="ones")
            nc.sync.dma_start(bqk_sb[:, :], bqk)
            bv_rep = pool_misc.tile([128, C], F32, tag="bvrep")
            nc.sync.dma_start(bv_sb[:, :], bv)
            nc.sync.dma_start(bo_sb[:, :], bo)
            nc.sync.dma_start(ones_sb[:, :], ones_row)
            nc.gpsimd.partition_broadcast(bv_rep[:, :], bv_sb[0:1, :].bitcast(F32))

            V_sb = [pool_V.tile([128, H * VW], F32R, tag=f"V{tb}", name=f"V{tb}") for tb in range(TB)]
            qk_sb = [pool_qk.tile([128, N], F32R, tag=f"qk{jb}", name=f"qk{jb}") for jb in range(JB)]

            # ======== phases A (V natural) and B (qkT) ========
            with tc.tile_pool(name="x", bufs=1) as pool_x, \
                 tc.tile_pool(name="ps_proj", bufs=6, space="PSUM") as ps_proj:

                x_sb = [pool_x.tile([128, N], F32R, tag=f"x{cb}", name=f"x{cb}") for cb in range(CB)]

                # ---- A: V[token, vfeat] ----
                with tc.tile_pool(name="wv", bufs=1) as pool_wv:
                    wv_sb = [pool_wv.tile([128, C], F32R, tag=f"wv{cb}", name=f"wv{cb}") for cb in range(CB)]
                    # load order: x fully first (both A and B need it), then wv
                    for cb in range(CB):
                        for ch in range(2):
                            nc.sync.dma_start(
                                x_sb[cb][:, ch * 512:(ch + 1) * 512],
                                xT[cb * 128:(cb + 1) * 128, ch * 512:(ch + 1) * 512])
                    for cb in range(CB):
                        for ch in range(2):
                            eng = nc.sync if (cb + ch) % 2 == 1 else nc.scalar
                            eng.dma_start(
                                wv_sb[cb][:, ch * 512:(ch + 1) * 512],
                                wvT[cb * 128:(cb + 1) * 128, ch * 512:(ch + 1) * 512])
                    # ones columns of V_hat groups (only needed by phase C's PV)
                    for tb in range(TB):
                        nc.sync.dma_start(V_sb[tb][:, 64::VW], ones_col)
                    for tb in range(TB):
                        for vc in range(2):
                            ps = ps_proj.tile([128, 512], F32, tag="psA")
                            for cb in range(CB):
                                nc.tensor.matmul(
                                    ps[:, :],
                                    x_sb[cb][:, tb * 128:(tb + 1) * 128],
                                    wv_sb[cb][:, vc * 512:(vc + 1) * 512],
                                    start=(cb == 0), stop=(cb == CB - 1),
                                )
                            # scatter 8 heads x 64 cols into the 65-strided
                            # layout, adding the broadcast v bias
                            dst = V_sb[tb][:, vc * 8 * VW:(vc + 1) * 8 * VW]
                            dst3 = dst.rearrange("p (h d) -> p h d", h=8)[:, :, 0:64]
                            src3 = ps[:, :].rearrange("p (h d) -> p h d", h=8)
                            bv3 = bv_rep[:, vc * 512:(vc + 1) * 512].rearrange(
                                "p (h d) -> p h d", h=8)
                            nc.vector.tensor_add(dst3, src3, bv3)

                # ---- B: qkT[feature, token] ----
                with tc.tile_pool(name="wqk", bufs=12) as pool_wqk:
                    # k-feature half (jh=1) first so attention pairs can start
                    # as soon as their q block lands in the second half
                    for jh in (1, 0):  # stream wqk in two 1024-feature halves
                        wqk_sb = []
                        for cb in range(CB):
                            t = pool_wqk.tile([128, C], F32R, tag="wqk", name="wqk")
                            nc.sync.dma_start(
                                t[:, :],
                                wqkT[cb * 128:(cb + 1) * 128, jh * C:(jh + 1) * C],
                            )
                            wqk_sb.append(t)
                        for jbl in range(8):
                            jb = jh * 8 + jbl
                            for nch in range(2):
                                ps = ps_proj.tile([128, 512], F32, tag="psA")
                                for cb in range(CB):
                                    nc.tensor.matmul(
                                        ps[:, :],
                                        wqk_sb[cb][:, jbl * 128:(jbl + 1) * 128],
                                        x_sb[cb][:, nch * 512:(nch + 1) * 512],
                                        start=(cb == 0), stop=(cb == CB - 1),
                                    )
                                nc.vector.tensor_scalar(
                                    qk_sb[jb][:, nch * 512:(nch + 1) * 512], ps[:, :],
                                    bqk_sb[:, jb:jb + 1], None, mybir.AluOpType.add,
                                )

            # ======== phases C (attention) and D (out projection) ========
            # A^T reuses the q-feature qk tiles: block hp's q/k data is dead
            # once pair hp's S^T matmuls are done.
            A_sb = qk_sb[0:8]
            with tc.tile_pool(name="wo", bufs=1) as pool_wo:
                wo_sb = [pool_wo.tile([128, C], F32R, tag=f"wo{cb}", name=f"wo{cb}") for cb in range(CB)]
                for cb in range(CB):
                    nc.sync.dma_start(wo_sb[cb][:, :], woT[cb * 128:(cb + 1) * 128, :])

                with tc.tile_pool(name="PT", bufs=8) as pool_PT, \
                     tc.tile_pool(name="norm", bufs=2) as pool_norm, \
                     tc.tile_pool(name="ps_S", bufs=2, space="PSUM") as ps_S, \
                     tc.tile_pool(name="ps_O", bufs=2, space="PSUM") as ps_O:

                    # ---- flat skewed pipeline over 128 (pair, kb, ic) units.
                    # Each unit packs BOTH heads of the pair into one S tile:
                    # cols 0:512 = head h0, cols 512:1024 = head h1. The two QK
                    # matmuls land on complementary PE row groups (0-63 /
                    # 64-127) so they run concurrently and keep the array's
                    # activity monitor at full clock; one exp covers both.
                    units = [(hp, kb, ic)
                             for hp in range(8) for kb in range(TB)
                             for ic in range(2)]

                    s_ps_of = {}
                    pt_of = {}
                    o_ps_of = {}

                    def emit_qk(u):
                        hp, kb, ic = u
                        s_ps = ps_S.tile([128, N], F32, tag="S", name="S")
                        for hh in range(2):
                            r0, r1 = hh * 64, hh * 64 + 64
                            nc.tensor.matmul(
                                s_ps[:, hh * 512:(hh + 1) * 512],
                                qk_sb[8 + hp][r0:r1, kb * 128:(kb + 1) * 128],
                                qk_sb[hp][r0:r1, ic * 512:(ic + 1) * 512],
                                start=True, stop=True,
                            )
                        s_ps_of[u] = s_ps

                    def emit_exp(u):
                        p_t = pool_PT.tile([128, N], F32R, tag="pt", name="pt")
                        nc.scalar.activation(p_t[:, :], s_ps_of.pop(u)[:, :], EXP,
                                             scale=float(SCALE))
                        pt_of[u] = p_t

                    def emit_pv(u):
                        hp, kb, ic = u
                        p_t = pt_of.pop(u)
                        for hh in range(2):
                            h = 2 * hp + hh
                            if kb == 0 and ic == 0:
                                o_ps_of[h] = ps_O.tile([VW, N], F32, tag="O",
                                                       name="O")
                            nc.tensor.matmul(
                                o_ps_of[h][:, ic * 512:(ic + 1) * 512],
                                V_sb[kb][:, h * VW:(h + 1) * VW],
                                p_t[:, hh * 512:(hh + 1) * 512],
                                start=(kb == 0), stop=(kb == TB - 1),
                            )

                    def emit_norm(h):
                        hp, hh = h // 2, h % 2
                        o_ps = o_ps_of.pop(h)
                        # one copy to SBUF frees the PSUM bank immediately so
                        # the next pair's PV accumulation can begin
                        o_cp = pool_norm.tile([VW, N], F32, tag="ocp", name="ocp",
                                              bufs=2)
                        nc.vector.tensor_copy(o_cp[:, :], o_ps[:, :])
                        s128 = pool_norm.tile([128, 8], F32, tag="s128", name="s128")
                        nc.sync.dma_start(s128[:, :], o_cp[64:65, :])
                        r128 = pool_norm.tile([128, 8], F32, tag="r128", name="r128")
                        nc.vector.reciprocal(r128[:, :], s128[:, :])
                        r0t = pool_norm.tile([1, N], F32, tag="r0", name="r0")
                        nc.sync.dma_start(r0t[0:1, :], r128[:, :])
                        r_rep = pool_norm.tile([64, N], F32, tag="rrep", name="rrep")
                        nc.gpsimd.partition_broadcast(r_rep[:, :], r0t[0:1, :])
                        if hh == 0:
                            nc.vector.tensor_mul(
                                A_sb[hp][0:64, :], o_cp[0:64, :], r_rep[:, :])
                        else:
                            a_tmp = pool_norm.tile([64, N], F32R, tag="atmp",
                                                   name="atmp")
                            nc.vector.tensor_mul(
                                a_tmp[:, :], o_cp[0:64, :], r_rep[:, :])
                            for ch in range(4):
                                nc.sync.dma_start(
                                    A_sb[hp][64:128, ch * 256:(ch + 1) * 256],
                                    a_tmp[:, ch * 256:(ch + 1) * 256])

                    SKEW = 2
                    for j in range(SKEW):
                        emit_qk(units[j])
                    for i, u in enumerate(units):
                        emit_exp(u)
                        if i + SKEW < len(units):
                            emit_qk(units[i + SKEW])
                        emit_pv(u)
                        if u[1] == TB - 1 and u[2] == 1:
                            emit_norm(2 * u[0])
                            emit_norm(2 * u[0] + 1)

            with tc.tile_pool(name="outp", bufs=4) as pool_out, \
                     tc.tile_pool(name="ps_out", bufs=6, space="PSUM") as ps_out:
                    for cb in range(CB):
                        for nch in range(2):
                            ps = ps_out.tile([128, 512], F32, tag="po")
                            for hb in range(8):
                                nc.tensor.matmul(
                                    ps[:, :],
                                    wo_sb[hb][:, cb * 128:(cb + 1) * 128],
                                    A_sb[hb][:, nch * 512:(nch + 1) * 512],
                                    start=(hb == 0), stop=(hb == 7),
                                )
                            o_t = pool_out.tile([128, 512], F32, tag="ot")
                            nc.vector.tensor_scalar(
                                o_t[:, :], ps[:, :], bo_sb[:, cb:cb + 1], None,
                                mybir.AluOpType.add,
                            )
                            for sh in range(2):
                                eng = nc.sync if (nch + sh) % 2 == 0 else nc.scalar
                                eng.dma_start(
                                    outT[cb * 128:(cb + 1) * 128,
                                         nch * 512 + sh * 256:
                                         nch * 512 + (sh + 1) * 256],
                                    o_t[:, sh * 256:(sh + 1) * 256],
                                )
    nc.compile()
    return nc


def _get_nc():
    if "nc" not in _COMPILED:
        _COMPILED["nc"] = _build()
    return _COMPILED["nc"]


def _run(x, in_proj_weight, in_proj_bias, out_proj_weight, out_proj_bias,
         trace=False):
    from concourse.bass_utils import run_bass_kernel_spmd

    nc = _get_nc()
    x = np.ascontiguousarray(np.asarray(x, dtype=np.float32))
    w_in = np.asarray(in_proj_weight, dtype=np.float32)
    b_in = np.asarray(in_proj_bias, dtype=np.float32)
    w_out = np.asarray(out_proj_weight, dtype=np.float32)
    b_out = np.asarray(out_proj_bias, dtype=np.float32)

    wqkT = np.ascontiguousarray(w_in[0:2 * C].T)          # [C, 2C]
    wvT = np.ascontiguousarray(w_in[2 * C:3 * C].T)       # [C, C]
    woT = np.ascontiguousarray(w_out.T)                   # [C, C]
    shared = {
        "wqkT": wqkT,
        "wvT": wvT,
        "woT": woT,
        "bqk": np.ascontiguousarray(b_in[0:2 * C].reshape(16, 128).T),
        "bv": np.ascontiguousarray(b_in[2 * C:3 * C])[None, :],
        "bo": np.ascontiguousarray(b_out.reshape(8, 128).T),
        "ones_col": np.ones((128, 16), dtype=np.float32),
        "ones_row": np.ones((1, 512), dtype=np.float32),
    }
    in_maps = []
    for c in range(NCORES):
        m = dict(shared)
        m["xT"] = np.ascontiguousarray(x[c].T)
        in_maps.append(m)

    res = run_bass_kernel_spmd(nc, in_maps, core_ids=list(range(NCORES)),
                               trace=trace)
    out = np.stack([
        np.ascontiguousarray(res.results[c]["outT"].T) for c in range(NCORES)
    ]).astype(np.float32)
    return out, res


def kernel(x, in_proj_weight, in_proj_bias, out_proj_weight, out_proj_bias):
    out, _ = _run(x, in_proj_weight, in_proj_bias, out_proj_weight,
                  out_proj_bias)
    return out


# revision 39
# speedup vs baseline: 1.0630x; 1.0630x over previous
"""Multi-head attention (B=8, N=1024, C=1024, H=16) on 8 TRN2 NeuronCores.

Strategy: pure data parallelism — each core computes one batch element with
replicated weights (no collectives). Per-core single-head-dim layout:

  inputs (host-prepped, transposed so every matmul contracts on partitions):
    xT   [C, N]      = x[b].T
    wqkT [C, 2C_qk]  = in_proj_weight[0:2048].T     (q then k features)
    wvT  [C, C]      = in_proj_weight[2048:3072].T
    woT  [C, C]      = out_proj_weight.T            ((h,d) rows, co cols)
  phases on-device (all matmuls in float32r: full-rate fp32, ~1e-3 rounding):
    A: V natural [token, vfeat] per 65-wide head group w/ ones column
       (the ones column makes the PV matmul also produce softmax row-sums)
    B: qkT [feature, token] (transposed q/k for QK^T)
    C: per head h: S^T[key, query] = K_h^T.T @ Q_h^T; P = exp(S*scale);
       O^T[d+1, query] = V_hat.T @ P^T (row 64 = softmax sums);
       normalize via reciprocal + gpsimd partition_broadcast
    D: outT [co, token] = woT.T @ A^T + bias

Output: outT per core, host transposes back and stacks.
"""
import numpy as np

B, N, C = 8, 1024, 1024
H = 16
HD = C // H               # 64
SCALE = HD ** (-0.5)
NCORES = 8

_COMPILED = {}


def _build():
    import concourse.bass as bass
    import concourse.tile as tile
    from concourse import bacc, mybir

    F32 = mybir.dt.float32
    F32R = mybir.dt.float32r
    BF16 = mybir.dt.bfloat16
    EXP = mybir.ActivationFunctionType.Exp

    nc = bacc.Bacc("TRN2", target_bir_lowering=False, debug=False)

    xT = nc.dram_tensor("xT", [C, N], BF16, kind="ExternalInput").ap()
    wqkT = nc.dram_tensor("wqkT", [C, 2 * C], BF16, kind="ExternalInput").ap()
    wvT = nc.dram_tensor("wvT", [C, C], BF16, kind="ExternalInput").ap()
    woT = nc.dram_tensor("woT", [C, C], F32R, kind="ExternalInput").ap()
    bqk = nc.dram_tensor("bqk", [128, 16], F32, kind="ExternalInput").ap()
    bv = nc.dram_tensor("bv", [1, C], F32R, kind="ExternalInput").ap()
    bo = nc.dram_tensor("bo", [128, 8], F32, kind="ExternalInput").ap()
    ones_col = nc.dram_tensor("ones_col", [128, 16], F32R, kind="ExternalInput").ap()
    ones_row = nc.dram_tensor("ones_row", [1, 512], F32R, kind="ExternalInput").ap()
    outT = nc.dram_tensor("outT", [C, N], F32, kind="ExternalOutput").ap()

    CB = C // 128      # 8 contraction blocks
    TB = N // 128      # 8 token blocks
    JB = 2 * C // 128  # 16 qk feature blocks
    VW = 65            # per-head V width (64 feats + ones col)

    with tile.TileContext(nc) as tc:
        with tc.tile_pool(name="misc", bufs=1) as pool_misc, \
             tc.tile_pool(name="V", bufs=1) as pool_V, \
             tc.tile_pool(name="qk", bufs=1) as pool_qk:

            bqk_sb = pool_misc.tile([128, 16], F32, tag="bqk")
            bv_sb = pool_misc.tile([1, C], F32R, tag="bv")
            bo_sb = pool_misc.tile([128, 8], F32, tag="bo")
            ones_sb = pool_misc.tile([1, 512], F32R, tag="ones")
            nc.sync.dma_start(bqk_sb[:, :], bqk)
            bv_rep = pool_misc.tile([128, C], F32, tag="bvrep")
            nc.sync.dma_start(bv_sb[:, :], bv)
            nc.sync.dma_start(bo_sb[:, :], bo)
            nc.sync.dma_start(ones_sb[:, :], ones_row)
            nc.gpsimd.partition_broadcast(bv_rep[:, :], bv_sb[0:1, :].bitcast(F32))

            V_sb = [pool_V.tile([128, H * VW], F32R, tag=f"V{tb}", name=f"V{tb}") for tb in range(TB)]
            qk_sb = [pool_qk.tile([128, N], F32R, tag=f"qk{jb}", name=f"qk{jb}") for jb in range(JB)]

            # ======== phases A (V natural) and B (qkT) ========
            with tc.tile_pool(name="x", bufs=1) as pool_x, \
                 tc.tile_pool(name="ps_proj", bufs=6, space="PSUM") as ps_proj:

                x_sb = [pool_x.tile([128, N], BF16, tag=f"x{cb}", name=f"x{cb}") for cb in range(CB)]

                # ---- A: V[token, vfeat] ----
                with tc.tile_pool(name="wv", bufs=1) as pool_wv:
                    wv_sb = [pool_wv.tile([128, C], BF16, tag=f"wv{cb}", name=f"wv{cb}") for cb in range(CB)]
                    # load order: x fully first (both A and B need it), then wv
                    for cb in range(CB):
                        for ch in range(2):
                            nc.sync.dma_start(
                                x_sb[cb][:, ch * 512:(ch + 1) * 512],
                                xT[cb * 128:(cb + 1) * 128, ch * 512:(ch + 1) * 512])
                    for cb in range(CB):
                        for ch in range(2):
                            eng = nc.sync if (cb + ch) % 2 == 1 else nc.scalar
                            eng.dma_start(
                                wv_sb[cb][:, ch * 512:(ch + 1) * 512],
                                wvT[cb * 128:(cb + 1) * 128, ch * 512:(ch + 1) * 512])
                    # ones columns of V_hat groups (only needed by phase C's PV)
                    for tb in range(TB):
                        nc.sync.dma_start(V_sb[tb][:, 64::VW], ones_col)
                    for tb in range(TB):
                        for vc in range(2):
                            ps = ps_proj.tile([128, 512], F32, tag="psA")
                            for cb in range(CB):
                                nc.tensor.matmul(
                                    ps[:, :],
                                    x_sb[cb][:, tb * 128:(tb + 1) * 128],
                                    wv_sb[cb][:, vc * 512:(vc + 1) * 512],
                                    start=(cb == 0), stop=(cb == CB - 1),
                                )
                            # scatter 8 heads x 64 cols into the 65-strided
                            # layout, adding the broadcast v bias
                            dst = V_sb[tb][:, vc * 8 * VW:(vc + 1) * 8 * VW]
                            dst3 = dst.rearrange("p (h d) -> p h d", h=8)[:, :, 0:64]
                            src3 = ps[:, :].rearrange("p (h d) -> p h d", h=8)
                            bv3 = bv_rep[:, vc * 512:(vc + 1) * 512].rearrange(
                                "p (h d) -> p h d", h=8)
                            nc.vector.tensor_add(dst3, src3, bv3)

                # ---- B: qkT[feature, token] ----
                with tc.tile_pool(name="wqk", bufs=12) as pool_wqk:
                    # k-feature half (jh=1) first so attention pairs can start
                    # as soon as their q block lands in the second half
                    for jh in (1, 0):  # stream wqk in two 1024-feature halves
                        wqk_sb = []
                        for cb in range(CB):
                            t = pool_wqk.tile([128, C], BF16, tag="wqk", name="wqk")
                            nc.sync.dma_start(
                                t[:, :],
                                wqkT[cb * 128:(cb + 1) * 128, jh * C:(jh + 1) * C],
                            )
                            wqk_sb.append(t)
                        for jbl in range(8):
                            jb = jh * 8 + jbl
                            for nch in range(2):
                                ps = ps_proj.tile([128, 512], F32, tag="psA")
                                for cb in range(CB):
                                    nc.tensor.matmul(
                                        ps[:, :],
                                        wqk_sb[cb][:, jbl * 128:(jbl + 1) * 128],
                                        x_sb[cb][:, nch * 512:(nch + 1) * 512],
                                        start=(cb == 0), stop=(cb == CB - 1),
                                    )
                                nc.vector.tensor_scalar(
                                    qk_sb[jb][:, nch * 512:(nch + 1) * 512], ps[:, :],
                                    bqk_sb[:, jb:jb + 1], None, mybir.AluOpType.add,
                                )

            # ======== phases C (attention) and D (out projection) ========
            # A^T reuses the q-feature qk tiles: block hp's q/k data is dead
            # once pair hp's S^T matmuls are done.
            A_sb = qk_sb[0:8]
            with tc.tile_pool(name="wo", bufs=1) as pool_wo:
                wo_sb = [pool_wo.tile([128, C], F32R, tag=f"wo{cb}", name=f"wo{cb}") for cb in range(CB)]
                for cb in range(CB):
                    nc.sync.dma_start(wo_sb[cb][:, :], woT[cb * 128:(cb + 1) * 128, :])

                with tc.tile_pool(name="PT", bufs=8) as pool_PT, \
                     tc.tile_pool(name="norm", bufs=2) as pool_norm, \
                     tc.tile_pool(name="ps_S", bufs=2, space="PSUM") as ps_S, \
                     tc.tile_pool(name="ps_O", bufs=2, space="PSUM") as ps_O:

                    # ---- flat skewed pipeline over 128 (pair, kb, ic) units.
                    # Each unit packs BOTH heads of the pair into one S tile:
                    # cols 0:512 = head h0, cols 512:1024 = head h1. The two QK
                    # matmuls land on complementary PE row groups (0-63 /
                    # 64-127) so they run concurrently and keep the array's
                    # activity monitor at full clock; one exp covers both.
                    units = [(hp, kb, ic)
                             for hp in range(8) for kb in range(TB)
                             for ic in range(2)]

                    s_ps_of = {}
                    pt_of = {}
                    o_ps_of = {}

                    def emit_qk(u):
                        hp, kb, ic = u
                        s_ps = ps_S.tile([128, N], F32, tag="S", name="S")
                        for hh in range(2):
                            r0, r1 = hh * 64, hh * 64 + 64
                            nc.tensor.matmul(
                                s_ps[:, hh * 512:(hh + 1) * 512],
                                qk_sb[8 + hp][r0:r1, kb * 128:(kb + 1) * 128],
                                qk_sb[hp][r0:r1, ic * 512:(ic + 1) * 512],
                                start=True, stop=True,
                            )
                        s_ps_of[u] = s_ps

                    def emit_exp(u):
                        p_t = pool_PT.tile([128, N], F32R, tag="pt", name="pt")
                        nc.scalar.activation(p_t[:, :], s_ps_of.pop(u)[:, :], EXP,
                                             scale=float(SCALE))
                        pt_of[u] = p_t

                    def emit_pv(u):
                        hp, kb, ic = u
                        p_t = pt_of.pop(u)
                        for hh in range(2):
                            h = 2 * hp + hh
                            if kb == 0 and ic == 0:
                                o_ps_of[h] = ps_O.tile([VW, N], F32, tag="O",
                                                       name="O")
                            nc.tensor.matmul(
                                o_ps_of[h][:, ic * 512:(ic + 1) * 512],
                                V_sb[kb][:, h * VW:(h + 1) * VW],
                                p_t[:, hh * 512:(hh + 1) * 512],
                                start=(kb == 0), stop=(kb == TB - 1),
                            )

                    def emit_norm(h):
                        hp, hh = h // 2, h % 2
                        o_ps = o_ps_of.pop(h)
                        # one copy to SBUF frees the PSUM bank immediately so
                        # the next pair's PV accumulation can begin
                        o_cp = pool_norm.tile([VW, N], F32, tag="ocp", name="ocp",
                                              bufs=2)
                        nc.vector.tensor_copy(o_cp[:, :], o_ps[:, :])
                        s128 = pool_norm.tile([128, 8], F32, tag="s128", name="s128")
                        nc.sync.dma_start(s128[:, :], o_cp[64:65, :])
                        r128 = pool_norm.tile([128, 8], F32, tag="r128", name="r128")
                        nc.vector.reciprocal(r128[:, :], s128[:, :])
                        r0t = pool_norm.tile([1, N], F32, tag="r0", name="r0")
                        nc.sync.dma_start(r0t[0:1, :], r128[:, :])
                        r_rep = pool_norm.tile([64, N], F32, tag="rrep", name="rrep")
                        nc.gpsimd.partition_broadcast(r_rep[:, :], r0t[0:1, :])
                        if hh == 0:
                            nc.vector.tensor_mul(
                                A_sb[hp][0:64, :], o_cp[0:64, :], r_rep[:, :])
                        else:
                            a_tmp = pool_norm.tile([64, N], F32R, tag="atmp",
                                                   name="atmp")
                            nc.vector.tensor_mul(
                                a_tmp[:, :], o_cp[0:64, :], r_rep[:, :])
                            for ch in range(4):
                                nc.sync.dma_start(
                                    A_sb[hp][64:128, ch * 256:(ch + 1) * 256],
                                    a_tmp[:, ch * 256:(ch + 1) * 256])

                    SKEW = 2
                    for j in range(SKEW):
                        emit_qk(units[j])
                    for i, u in enumerate(units):
                        emit_exp(u)
                        if i + SKEW < len(units):
                            emit_qk(units[i + SKEW])
                        emit_pv(u)
                        if u[1] == TB - 1 and u[2] == 1:
                            emit_norm(2 * u[0])
                            emit_norm(2 * u[0] + 1)

            with tc.tile_pool(name="outp", bufs=4) as pool_out, \
                     tc.tile_pool(name="ps_out", bufs=6, space="PSUM") as ps_out:
                    for cb in range(CB):
                        for nch in range(2):
                            ps = ps_out.tile([128, 512], F32, tag="po")
                            for hb in range(8):
                                nc.tensor.matmul(
                                    ps[:, :],
                                    wo_sb[hb][:, cb * 128:(cb + 1) * 128],
                                    A_sb[hb][:, nch * 512:(nch + 1) * 512],
                                    start=(hb == 0), stop=(hb == 7),
                                )
                            o_t = pool_out.tile([128, 512], F32, tag="ot")
                            nc.vector.tensor_scalar(
                                o_t[:, :], ps[:, :], bo_sb[:, cb:cb + 1], None,
                                mybir.AluOpType.add,
                            )
                            for sh in range(2):
                                eng = nc.sync if (nch + sh) % 2 == 0 else nc.scalar
                                eng.dma_start(
                                    outT[cb * 128:(cb + 1) * 128,
                                         nch * 512 + sh * 256:
                                         nch * 512 + (sh + 1) * 256],
                                    o_t[:, sh * 256:(sh + 1) * 256],
                                )
    nc.compile()
    return nc


def _get_nc():
    if "nc" not in _COMPILED:
        _COMPILED["nc"] = _build()
    return _COMPILED["nc"]


def _run(x, in_proj_weight, in_proj_bias, out_proj_weight, out_proj_bias,
         trace=False):
    from concourse.bass_utils import run_bass_kernel_spmd

    nc = _get_nc()
    x = np.ascontiguousarray(np.asarray(x, dtype=np.float32))
    w_in = np.asarray(in_proj_weight, dtype=np.float32)
    b_in = np.asarray(in_proj_bias, dtype=np.float32)
    w_out = np.asarray(out_proj_weight, dtype=np.float32)
    b_out = np.asarray(out_proj_bias, dtype=np.float32)

    import ml_dtypes
    bf16 = ml_dtypes.bfloat16
    wqkT = np.ascontiguousarray(w_in[0:2 * C].T).astype(bf16)   # [C, 2C]
    wvT = np.ascontiguousarray(w_in[2 * C:3 * C].T).astype(bf16)  # [C, C]
    woT = np.ascontiguousarray(w_out.T)                   # [C, C]
    shared = {
        "wqkT": wqkT,
        "wvT": wvT,
        "woT": woT,
        "bqk": np.ascontiguousarray(b_in[0:2 * C].reshape(16, 128).T),
        "bv": np.ascontiguousarray(b_in[2 * C:3 * C])[None, :],
        "bo": np.ascontiguousarray(b_out.reshape(8, 128).T),
        "ones_col": np.ones((128, 16), dtype=np.float32),
        "ones_row": np.ones((1, 512), dtype=np.float32),
    }
    in_maps = []
    for c in range(NCORES):
        m = dict(shared)
        m["xT"] = np.ascontiguousarray(x[c].T).astype(bf16)
        in_maps.append(m)

    res = run_bass_kernel_spmd(nc, in_maps, core_ids=list(range(NCORES)),
                               trace=trace)
    out = np.stack([
        np.ascontiguousarray(res.results[c]["outT"].T) for c in range(NCORES)
    ]).astype(np.float32)
    return out, res


def kernel(x, in_proj_weight, in_proj_bias, out_proj_weight, out_proj_bias):
    out, _ = _run(x, in_proj_weight, in_proj_bias, out_proj_weight,
                  out_proj_bias)
    return out
